# revision 8
# baseline (speedup 1.0000x reference)
"""AttnBlock (GroupNorm -> QKV 1x1 -> spatial attention -> proj_out -> residual)
for Trainium2, sharded over 8 NeuronCores.

Sharding: (batch b in {0,1}) x (4 query chunks of 1024 of the 4096 spatial
positions). Every core runs the same program; per-core inputs are column-
rotated so the core's query block sits at columns 0..1023 (attention is
permutation-invariant over key positions, GroupNorm stats over column order).

Layouts avoid all on-chip transposes:
  hn/K/Q:  [channel partitions, position free]
  V, P:    [position partitions, channel/query free]  (V^T = hn^T @ Wv^T via
           matmul with hn as the stationary operand)
Softmax row sums come from a ones-vector matmul; normalization is applied
after the P@V matmul (cheap: divides [512, 1024] instead of [4096, 1024]).
Matmuls run as float32r (full-rate fp32 path); V/P are stored bf16 for SBUF.
"""

import sys

sys.path.insert(0, "/opt/trn_rl_repo")

import numpy as np

C = 512
N = 4096  # h*w
NP = 4  # channel tiles of 128
QCH = 1024  # queries per core
EPS = 1e-6
GSIZE = 16  # channels per group
GELEMS = float(GSIZE * N)  # elements per group

_NC_CACHE = {}


def _build_nc(reps=1):
    import concourse.bacc as bacc
    import concourse.tile as tile
    from concourse import mybir

    dt = mybir.dt
    f32 = dt.float32
    f32r = dt.float32r
    bf16 = dt.bfloat16

    nc = bacc.Bacc("TRN2", target_bir_lowering=False, debug=False, num_devices=8)

    xb = nc.dram_tensor("xb", [C, N], f32, kind="ExternalInput").ap()
    wkT = nc.dram_tensor("wkT", [C, C], f32r, kind="ExternalInput").ap()
    wvT = nc.dram_tensor("wvT", [C, C], f32r, kind="ExternalInput").ap()
    wqTs = nc.dram_tensor("wqTs", [C, C], f32r, kind="ExternalInput").ap()
    woT = nc.dram_tensor("woT", [C, C], f32r, kind="ExternalInput").ap()
    bk_col = nc.dram_tensor("bk_col", [C, 1], f32, kind="ExternalInput").ap()
    bv_row = nc.dram_tensor("bv_row", [1, C], f32, kind="ExternalInput").ap()
    bqs_col = nc.dram_tensor("bqs_col", [C, 1], f32, kind="ExternalInput").ap()
    bo_col = nc.dram_tensor("bo_col", [C, 1], f32, kind="ExternalInput").ap()
    gamma_col = nc.dram_tensor("gamma_col", [C, 1], f32, kind="ExternalInput").ap()
    beta_col = nc.dram_tensor("beta_col", [C, 1], f32, kind="ExternalInput").ap()
    g8 = nc.dram_tensor("g8", [128, 8], f32, kind="ExternalInput").ap()
    e8 = nc.dram_tensor("e8", [8, 128], f32, kind="ExternalInput").ap()
    y = nc.dram_tensor("y", [C, QCH], f32, kind="ExternalOutput").ap()

    with tile.TileContext(nc) as tc:
        with (
            tc.tile_pool(name="kpool", bufs=1) as kpool,      # K: 4 x [128,4096] f32
            tc.tile_pool(name="vpool", bufs=1) as vpool,      # V^T: 32 x [128,512] bf16
            tc.tile_pool(name="qpool", bufs=1) as qpool,      # Q: 4 x [128,1024] f32
            tc.tile_pool(name="wkv", bufs=1) as wkv,          # wkT+wvT tiles
            tc.tile_pool(name="wx", bufs=4) as wx,            # wqTs then woT (shared slots)
            tc.tile_pool(name="xc", bufs=6) as xcp,           # streamed x chunks [128,512]
            tc.tile_pool(name="hn", bufs=4) as hnp,           # hn chunks [128,512]
            tc.tile_pool(name="scr", bufs=2) as scr,          # square scratch
            tc.tile_pool(name="pt", bufs=4) as ptp,           # exp(P) tiles bf16
            tc.tile_pool(name="att", bufs=4) as attp,         # attn output sbuf
            tc.tile_pool(name="ysb", bufs=4) as ysbp,         # final out tiles
            tc.tile_pool(name="xq", bufs=4) as xqp,           # residual chunks
            tc.tile_pool(name="small", bufs=1) as small,      # stats/bias vectors
            tc.tile_pool(name="ps", bufs=2, space="PSUM") as ps,
            tc.tile_pool(name="pv", bufs=4, space="PSUM") as pvp,
            tc.tile_pool(name="rs", bufs=1, space="PSUM") as rsp,
        ):
            # ---- persistent small tensors ----
            wk_t = [wkv.tile([128, C], f32r, tag=f"wk{p}", name=f"wk_t{p}") for p in range(NP)]
            wv_t = [wkv.tile([128, C], f32r, tag=f"wv{p}", name=f"wv_t{p}") for p in range(NP)]
            for p in range(NP):
                nc.sync.dma_start(wk_t[p][:], wkT[p * 128:(p + 1) * 128, :])
                nc.sync.dma_start(wv_t[p][:], wvT[p * 128:(p + 1) * 128, :])
            bk_t = [small.tile([128, 1], f32, tag=f"bk{p}", name=f"bk{p}") for p in range(NP)]
            bqs_t = [small.tile([128, 1], f32, tag=f"bqs{p}", name=f"bqs{p}") for p in range(NP)]
            bo_t = [small.tile([128, 1], f32, tag=f"bo{p}", name=f"bo{p}") for p in range(NP)]
            gam_t = [small.tile([128, 1], f32, tag=f"gam{p}", name=f"gam{p}") for p in range(NP)]
            bet_t = [small.tile([128, 1], f32, tag=f"bet{p}", name=f"bet{p}") for p in range(NP)]
            for p in range(NP):
                sl = slice(p * 128, (p + 1) * 128)
                nc.sync.dma_start(bk_t[p][:], bk_col[sl, :])
                nc.sync.dma_start(bqs_t[p][:], bqs_col[sl, :])
                nc.sync.dma_start(bo_t[p][:], bo_col[sl, :])
                nc.sync.dma_start(gam_t[p][:], gamma_col[sl, :])
                nc.sync.dma_start(bet_t[p][:], beta_col[sl, :])
            bv_t = small.tile([1, C], f32, tag="bv")
            nc.sync.dma_start(bv_t[:], bv_row[:])
            bv_bc = small.tile([128, C], f32, tag="bv_bc")
            nc.gpsimd.partition_broadcast(bv_bc[:], bv_t[:])
            g8_t = small.tile([128, 8], f32, tag="g8")
            nc.sync.dma_start(g8_t[:], g8[:])
            e8_t = small.tile([8, 128], f32, tag="e8")
            nc.sync.dma_start(e8_t[:], e8[:])
            ones_t = small.tile([128, 1], bf16, tag="ones")
            nc.vector.memset(ones_t[:], 1.0)

            for _rep in range(reps):
                # ================= GroupNorm statistics =================
                # per-channel sum / sum-of-squares, streamed in [128,512] chunks
                stc1 = [small.tile([128, 8], f32, tag=f"stc1_{p}", name=f"stc1_{p}") for p in range(NP)]
                stc2 = [small.tile([128, 8], f32, tag=f"stc2_{p}", name=f"stc2_{p}") for p in range(NP)]
                st = [small.tile([128, 2], f32, tag=f"st{p}", name=f"st{p}") for p in range(NP)]
                for p in range(NP):
                    for jb in range(8):
                        xt = xcp.tile([128, 512], f32, tag="xchunk")
                        nc.sync.dma_start(
                            xt[:], xb[p * 128:(p + 1) * 128, jb * 512:(jb + 1) * 512]
                        )
                        nc.vector.reduce_sum(
                            stc1[p][:, jb:jb + 1], xt[:], axis=mybir.AxisListType.X
                        )
                        sq = scr.tile([128, 512], f32, tag="sq")
                        nc.scalar.activation(
                            sq[:], xt[:], mybir.ActivationFunctionType.Square,
                            accum_out=stc2[p][:, jb:jb + 1],
                        )
                    nc.vector.reduce_sum(
                        st[p][:, 0:1], stc1[p][:], axis=mybir.AxisListType.X
                    )
                    nc.vector.reduce_sum(
                        st[p][:, 1:2], stc2[p][:], axis=mybir.AxisListType.X
                    )

                # group means via one-hot matmul: pg[p] = G8.T @ st[p] -> [8,2]
                # (G8 scaled 1/(16*4096) so cols are (mean, E[x^2]) per group)
                scale_t = [small.tile([128, 1], f32, tag=f"scale{p}", name=f"scale{p}") for p in range(NP)]
                shift_t = [small.tile([128, 1], f32, tag=f"shift{p}", name=f"shift{p}") for p in range(NP)]
                for p in range(NP):
                    pg = ps.tile([8, 2], f32, tag="pp")
                    nc.tensor.matmul(pg[:], g8_t[:], st[p][:], start=True, stop=True)
                    pgs = small.tile([8, 2], f32, tag="pgs")
                    nc.vector.tensor_copy(pgs[:], pg[:])
                    tmp = small.tile([8, 4], f32, tag="gtmp")
                    # var = E[x^2] - mean^2 ; rstd = 1/sqrt(var+eps)
                    nc.vector.tensor_mul(tmp[:, 0:1], pgs[:, 0:1], pgs[:, 0:1])
                    nc.vector.tensor_sub(tmp[:, 1:2], pgs[:, 1:2], tmp[:, 0:1])
                    nc.vector.tensor_scalar_add(tmp[:, 1:2], tmp[:, 1:2], EPS)
                    nc.scalar.activation(
                        tmp[:, 2:3], tmp[:, 1:2],
                        mybir.ActivationFunctionType.Sqrt,
                    )
                    rb = small.tile([8, 2], f32, tag="rb")
                    nc.vector.reciprocal(rb[:, 0:1], tmp[:, 2:3])
                    # -mean * rstd
                    nc.vector.tensor_mul(tmp[:, 3:4], pgs[:, 0:1], rb[:, 0:1])
                    nc.vector.tensor_scalar_mul(rb[:, 1:2], tmp[:, 3:4], -1.0)
                    # broadcast back to channels: pc[p] = E8.T @ rb -> [128,2]
                    pc = ps.tile([128, 2], f32, tag="pp")
                    nc.tensor.matmul(pc[:], e8_t[:], rb[:], start=True, stop=True)
                    nc.vector.tensor_mul(scale_t[p][:], gam_t[p][:], pc[:, 0:1])
                    tsh = small.tile([128, 1], f32, tag="tsh")
                    nc.vector.tensor_mul(tsh[:], gam_t[p][:], pc[:, 1:2])
                    nc.vector.tensor_add(shift_t[p][:], bet_t[p][:], tsh[:])

                # ================= projections (streamed over 8 col chunks) ====
                k_sb = [kpool.tile([128, N], f32r, tag=f"k{m}", name=f"k{m}") for m in range(NP)]
                vt_sb = [vpool.tile([128, C], bf16, tag=f"vt{j}", name=f"vt{j}") for j in range(32)]
                q_sb = [qpool.tile([128, QCH], f32r, tag=f"q{m}", name=f"q{m}") for m in range(NP)]
                wq_t = [wx.tile([128, C], f32r, tag="wx", name="wx_t") for _ in range(NP)]
                for p in range(NP):
                    nc.sync.dma_start(wq_t[p][:], wqTs[p * 128:(p + 1) * 128, :])

                for jb in range(8):
                    jsl = slice(jb * 512, (jb + 1) * 512)
                    hn_c = []
                    for p in range(NP):
                        xt = xcp.tile([128, 512], f32, tag="xchunk")
                        nc.sync.dma_start(xt[:], xb[p * 128:(p + 1) * 128, jsl])
                        hn = hnp.tile([128, 512], f32r, tag="hn")
                        nc.scalar.activation(
                            hn[:], xt[:], mybir.ActivationFunctionType.Identity,
                            bias=shift_t[p][:], scale=scale_t[p][:],
                        )
                        hn_c.append(hn)
                    # K chunk: [c_out tile m, 512 cols]
                    for m in range(NP):
                        msl = slice(m * 128, (m + 1) * 128)
                        pk = ps.tile([128, 512], f32, tag="pp")
                        for p in range(NP):
                            nc.tensor.matmul(
                                pk[:],
                                wk_t[p][:, msl],
                                hn_c[p][:],
                                start=(p == 0), stop=(p == NP - 1),
                            )
                        nc.scalar.activation(
                            k_sb[m][:, jsl], pk[:],
                            mybir.ActivationFunctionType.Identity,
                            bias=bk_t[m][:],
                        )
                    # V^T rows (4 j-tiles of 128 in this chunk): [j tile, c_out]
                    for jt in range(4):
                        jj = jb * 4 + jt
                        pvt = ps.tile([128, 512], f32, tag="pp")
                        for p in range(NP):
                            nc.tensor.matmul(
                                pvt[:],
                                hn_c[p][:, jt * 128:(jt + 1) * 128],
                                wv_t[p][:],
                                start=(p == 0), stop=(p == NP - 1),
                            )
                        nc.vector.tensor_add(vt_sb[jj][:], pvt[:], bv_bc[:])
                    # Q (only first two chunks = this core's query block)
                    if jb < 2:
                        for m in range(NP):
                            msl = slice(m * 128, (m + 1) * 128)
                            pq = ps.tile([128, 512], f32, tag="pp")
                            for p in range(NP):
                                nc.tensor.matmul(
                                    pq[:],
                                    wq_t[p][:, msl],
                                    hn_c[p][:],
                                    start=(p == 0), stop=(p == NP - 1),
                                )
                            nc.scalar.activation(
                                q_sb[m][:, jsl], pq[:],
                                mybir.ActivationFunctionType.Identity,
                                bias=bqs_t[m][:],
                            )

                # ================= attention =================
                wo_t = [wx.tile([128, C], f32r, tag="wx", name="wx_t") for _ in range(NP)]
                for p in range(NP):
                    nc.sync.dma_start(wo_t[p][:], woT[p * 128:(p + 1) * 128, :])

                for ci in range(2):
                    isl = slice(ci * 512, (ci + 1) * 512)
                    pv_ps = [pvp.tile([128, 512], f32, tag="pv", name="pv_ps") for _ in range(NP)]
                    rs_ps = rsp.tile([1, 512], f32, tag="rs")
                    for jt in range(32):
                        st_ps = ps.tile([128, 512], f32, tag="pp")
                        for p in range(NP):
                            nc.tensor.matmul(
                                st_ps[:],
                                k_sb[p][:, jt * 128:(jt + 1) * 128],
                                q_sb[p][:, isl],
                                start=(p == 0), stop=(p == NP - 1),
                            )
                        pt = ptp.tile([128, 512], bf16, tag="pt")
                        nc.scalar.activation(
                            pt[:], st_ps[:], mybir.ActivationFunctionType.Exp
                        )
                        nc.tensor.matmul(
                            rs_ps[:], ones_t[:], pt[:],
                            start=(jt == 0), stop=(jt == 31),
                        )
                        for m in range(NP):
                            nc.tensor.matmul(
                                pv_ps[m][:],
                                vt_sb[jt][:, m * 128:(m + 1) * 128],
                                pt[:],
                                start=(jt == 0), stop=(jt == 31),
                            )
                    recip = small.tile([1, 512], f32, tag="recip")
                    nc.vector.reciprocal(recip[:], rs_ps[:])
                    recip_bc = small.tile([128, 512], f32, tag="recip_bc")
                    nc.gpsimd.partition_broadcast(recip_bc[:], recip[:])
                    att = []
                    for m in range(NP):
                        a = attp.tile([128, 512], f32r, tag="att", name="att_t")
                        nc.vector.tensor_mul(a[:], pv_ps[m][:], recip_bc[:])
                        att.append(a)
                    # proj_out + bias + residual
                    for m in range(NP):
                        msl = slice(m * 128, (m + 1) * 128)
                        po = ps.tile([128, 512], f32, tag="pp")
                        for p in range(NP):
                            nc.tensor.matmul(
                                po[:],
                                wo_t[p][:, msl],
                                att[p][:],
                                start=(p == 0), stop=(p == NP - 1),
                            )
                        yt = ysbp.tile([128, 512], f32, tag="ysb")
                        nc.scalar.activation(
                            yt[:], po[:], mybir.ActivationFunctionType.Identity,
                            bias=bo_t[m][:],
                        )
                        xq_t = xqp.tile([128, 512], f32, tag="xq")
                        nc.sync.dma_start(xq_t[:], xb[msl, isl])
                        nc.vector.tensor_add(yt[:], yt[:], xq_t[:])
                        nc.sync.dma_start(y[msl, isl], yt[:])

    nc.compile()
    return nc


def get_nc(reps=1):
    if reps not in _NC_CACHE:
        _NC_CACHE[reps] = _build_nc(reps)
    return _NC_CACHE[reps]


def make_in_maps(x, gn_gamma, gn_beta, wq, bq, wk, bk, wv, bv, wo, bo):
    s = 1.0 / np.sqrt(C)
    shared = {
        "wkT": np.ascontiguousarray(wk.T, np.float32),
        "wvT": np.ascontiguousarray(wv.T, np.float32),
        "wqTs": np.ascontiguousarray(wq.T * s, np.float32),
        "woT": np.ascontiguousarray(wo.T, np.float32),
        "bk_col": np.ascontiguousarray(bk[:, None], np.float32),
        "bv_row": np.ascontiguousarray(bv[None, :], np.float32),
        "bqs_col": np.ascontiguousarray((bq * s)[:, None], np.float32),
        "bo_col": np.ascontiguousarray(bo[:, None], np.float32),
        "gamma_col": np.ascontiguousarray(gn_gamma[:, None], np.float32),
        "beta_col": np.ascontiguousarray(gn_beta[:, None], np.float32),
    }
    g8 = np.zeros((128, 8), np.float32)
    for i in range(128):
        g8[i, i // GSIZE] = 1.0 / GELEMS
    e8 = np.zeros((8, 128), np.float32)
    for i in range(128):
        e8[i // GSIZE, i] = 1.0
    shared["g8"] = g8
    shared["e8"] = e8

    xf = np.asarray(x, np.float32).reshape(2, C, N)
    in_maps = []
    for cid in range(8):
        bi, qc = cid // 4, cid % 4
        xb = np.ascontiguousarray(np.roll(xf[bi], -qc * QCH, axis=1))
        in_maps.append({"xb": xb, **shared})
    return in_maps


def kernel(**inputs):
    from concourse.bass_utils import run_bass_kernel_spmd

    x = np.asarray(inputs["x"], np.float32)
    in_maps = make_in_maps(
        x, inputs["gn_gamma"], inputs["gn_beta"],
        inputs["wq"], inputs["bq"], inputs["wk"], inputs["bk"],
        inputs["wv"], inputs["bv"], inputs["wo"], inputs["bo"],
    )
    nc = get_nc(reps=1)
    res = run_bass_kernel_spmd(nc, in_maps, core_ids=list(range(8)), trace=False)
    out = np.empty((2, C, N), np.float32)
    for cid in range(8):
        bi, qc = cid // 4, cid % 4
        out[bi][:, qc * QCH:(qc + 1) * QCH] = res.results[cid]["y"]
    return out.reshape(2, C, 64, 64)


if __name__ == "__main__":
    rng = np.random.default_rng(0)
    inputs = {
        "x": rng.standard_normal((2, C, 64, 64), np.float32),
        "gn_gamma": np.ones(C, np.float32),
        "gn_beta": np.zeros(C, np.float32),
    }
    s = 1.0 / np.sqrt(C)
    for nm in ("q", "k", "v", "o"):
        inputs[f"w{nm}"] = rng.standard_normal((C, C), np.float32) * s
        inputs[f"b{nm}"] = rng.standard_normal(C, np.float32) * 0.01
    out = kernel(**inputs)
    print("kernel ran, out shape", out.shape, "mean", out.mean())
